# revision 1
# baseline (speedup 1.0000x reference)
"""Trainium2 Bass kernel for nn_DivrocLoss (trilinear splat histogram + Huber loss).

Strategy (8 NeuronCores, SPMD):
  - Spatial sharding over the 256-slab z axis: core c owns slabs [32c, 32c+32).
    Each (point, volume) pair becomes ONE record carrying its grid-space y, x
    coords and BOTH z-tap weights (w0 = 1-fz for slab z0, w1 = fz for slab
    z0+1, with the pred/gt sign folded into the weights); records are binned
    by (z0 slab -> core, y-half, x-half).
    Records whose z-taps straddle a core boundary are split into two
    single-tap records. Boundary slabs receive the split halves, so their
    bins get a larger static cap (NB0) than interior slabs (NBI).
  - On device, each core processes z0-groups in slab order with rotating PSUM
    banks: a batch of 128 records builds its (negated) y-tent and x-tent
    bf16 [128,128] tiles ONCE (DVE iota-subtract + ACT Abs + DVE sub/min),
    then two weighted stationaries a0 = tentY*w0, a1 = tentY*w1 feed two PE
    matmuls accumulating into slab s and slab s+1 banks. Window-straddling
    y/x tap pairs are duplicated into both windows by the host; the
    window-local tents pick up exactly the in-window taps and out-of-grid
    taps vanish (grid_sample zero-padding semantics).
  - Signed weights accumulate the difference volume d directly; each PSUM
    tile sees one contiguous matmul accumulation group (mandatory on HW),
    and slab evacuation sums the slab's two phase-tiles and computes fused
    Huber partial sums Huber(d) = 0.5*|d|^2 - 0.5*relu(|d|-1)^2 via
    activation accumulate.
  - Host sums the 8 cores' [128, 64] partial-sum tiles.
"""

import sys

sys.path.insert(0, "/opt/trn_rl_repo")

import numpy as np
import ml_dtypes

from concourse import bacc, bass, mybir
import concourse.tile as tile
from concourse.bass_utils import run_bass_kernel_spmd

GRID = 256
CORES = 8
SLABS = 32  # slabs per core
YHALVES = 2
XHALVES = 2
NQ = YHALVES * XHALVES  # 4 (yh, xh) combos per slab position
NB0 = 67  # batches per group at slab position 0 (receives straddle halves)
NBI = 35  # batches per group at interior slab positions
NBS = [NB0] + [NBI] * (SLABS - 1)
TOT = NQ * sum(NBS)  # total batch-columns per core (4992)

F32 = mybir.dt.float32
BF16 = mybir.dt.bfloat16


def _group_col_offsets():
    offs = np.zeros(SLABS * NQ, dtype=np.int64)
    col = 0
    for s in range(SLABS):
        for q in range(NQ):
            offs[s * NQ + q] = col
            col += NBS[s]
    assert col == TOT
    return offs


def _prepare_shards(registration_pred, registration_gt, coords):
    """Build per-core [128, TOT] f32 arrays Y, X, W0, W1 of z-pair records."""
    ys, xs, w0s, w1s, bins = [], [], [], [], []
    for vol, reg in ((0, registration_pred), (1, registration_gt)):
        p = coords.astype(np.float32) + reg.astype(np.float32)
        # mirror the reference's exact f32 expression ((g+1)*size - 1) * 0.5
        g = ((p + np.float32(1.0)) * np.float32(GRID) - np.float32(1.0)) * np.float32(
            0.5
        )
        gx = g[:, 0]
        gy = g[:, 1]
        gz = g[:, 2]
        z0 = np.floor(gz)
        fz = (gz - z0).astype(np.float32)
        z0 = z0.astype(np.int64)
        sign = np.float32(1.0 if vol == 0 else -1.0)
        w0 = (1.0 - fz).astype(np.float32) * sign
        w1 = fz * sign
        # z0 == -1: only the z=0 tap is valid -> shift record to z0=0
        shift = z0 == -1
        w0 = np.where(shift, fz * sign, w0)
        w1 = np.where(shift, 0.0, w1)
        z0 = np.where(shift, 0, z0)
        # z0 == 255: the z=256 tap is out of grid
        w1 = np.where(z0 == GRID - 1, 0.0, w1)
        keep = (z0 >= 0) & (z0 <= GRID - 1)
        z0k, gyk, gxk = z0[keep], gy[keep], gx[keep]
        w0k, w1k = w0[keep], w1[keep]
        # split records whose taps straddle a core boundary
        strad = ((z0k % SLABS) == SLABS - 1) & (z0k < GRID - 1)
        w1a = np.where(strad, 0.0, w1k)
        recs = [
            (z0k, gyk, gxk, w0k, w1a),
            (z0k[strad] + 1, gyk[strad], gxk[strad], w1k[strad], np.zeros(strad.sum(), np.float32)),
        ]
        for zz, gyv, gxv, rw0, rw1 in recs:
            y0 = np.floor(gyv)
            x0 = np.floor(gxv)
            yh = np.clip(y0 // 128, 0, 1).astype(np.int64)
            xh = np.clip(x0 // 128, 0, 1).astype(np.int64)
            # duplicate window-straddling y/x tap pairs into the upper window
            dupy = y0 == 127
            dupx = x0 == 127
            dupyx = dupy & dupx
            for sel, byh, bxh in (
                (slice(None), yh, xh),
                (dupy, 1, xh[dupy]),
                (dupx, yh[dupx], 1),
                (dupyx, 1, 1),
            ):
                ys.append(gyv[sel])
                xs.append(gxv[sel])
                w0s.append(rw0[sel])
                w1s.append(rw1[sel])
                bins.append((zz[sel] * 2 + byh) * 2 + bxh)
    Y = np.concatenate(ys)
    X = np.concatenate(xs)
    W0 = np.concatenate(w0s)
    W1 = np.concatenate(w1s)
    B = np.concatenate(bins)  # global bin in [0, 1024)

    order = np.argsort(B, kind="stable")
    Y, X, W0, W1, B = Y[order], X[order], W0[order], W1[order], B[order]
    nbins = GRID * NQ
    counts = np.bincount(B, minlength=nbins)

    offs_core = _group_col_offsets()  # per (slab_pos, q) within-core col offset
    zz = np.arange(GRID)
    core_of = zz // SLABS
    pos_of = zz % SLABS
    bin_caps = np.repeat(np.array(NBS)[pos_of] * 128, NQ)
    if (counts > bin_caps).any():
        raise RuntimeError("bin overflow")
    # global column offset per bin
    bin_cols = (
        core_of.repeat(NQ) * TOT
        + offs_core[(pos_of.repeat(NQ) * NQ) + np.tile(np.arange(NQ), GRID)]
    )

    starts = np.zeros(nbins + 1, dtype=np.int64)
    np.cumsum(counts, out=starts[1:])
    rank = np.arange(len(B), dtype=np.int64) - starts[B]
    dest = bin_cols[B] * 128 + rank

    def field_tiles(vals):
        flat = np.zeros(CORES * TOT * 128, dtype=np.float32)
        flat[dest] = vals
        out = []
        for c in range(CORES):
            block = flat[c * TOT * 128 : (c + 1) * TOT * 128]
            out.append(np.ascontiguousarray(block.reshape(TOT, 128).T))
        return out

    return list(
        zip(field_tiles(Y), field_tiles(X), field_tiles(W0), field_tiles(W1))
    )


def _sb_chunks(nb):
    """Split nb batches into superblocks of up to 16."""
    out = []
    i = 0
    while i < nb:
        sz = min(16, nb - i)
        out.append((i, sz))
        i += sz
    return out


def _build_program():
    nc = bacc.Bacc("TRN2", target_bir_lowering=False, debug=False, num_devices=CORES)
    Yd = nc.declare_dram_parameter("Y", [128, TOT], F32, isOutput=False)
    Xd = nc.declare_dram_parameter("X", [128, TOT], F32, isOutput=False)
    W0d = nc.declare_dram_parameter("W0", [128, TOT], F32, isOutput=False)
    W1d = nc.declare_dram_parameter("W1", [128, TOT], F32, isOutput=False)
    IOTAd = nc.declare_dram_parameter("IOTA", [128, GRID], BF16, isOutput=False)
    OUTd = nc.declare_dram_parameter("OUT", [128, 2 * SLABS], F32, isOutput=True)

    AluOp = mybir.AluOpType
    Act = mybir.ActivationFunctionType
    offs_core = _group_col_offsets()

    with tile.TileContext(nc) as tc:
        with (
            tc.tile_pool(name="persist", bufs=1) as persist,
            tc.tile_pool(name="eab", bufs=4) as eab,
            tc.tile_pool(name="tab", bufs=3) as tab,
            tc.tile_pool(name="atile", bufs=12) as atile,
            tc.tile_pool(name="evac", bufs=2) as evac,
            tc.tile_pool(name="psum", bufs=8, space="PSUM") as psum,
        ):
            y_t = persist.tile([128, TOT], F32, tag="yt")
            nc.sync.dma_start(out=y_t[:], in_=Yd[:])
            x_t = persist.tile([128, TOT], F32, tag="xt")
            nc.sync.dma_start(out=x_t[:], in_=Xd[:])
            w0_t = persist.tile([128, TOT], F32, tag="w0t")
            nc.sync.dma_start(out=w0_t[:], in_=W0d[:])
            w1_t = persist.tile([128, TOT], F32, tag="w1t")
            nc.sync.dma_start(out=w1_t[:], in_=W1d[:])
            iota_t = persist.tile([128, GRID], BF16, tag="iota")
            nc.sync.dma_start(out=iota_t[:], in_=IOTAd[:])
            xn_t = persist.tile([128, TOT], F32, tag="xnt")
            nc.vector.tensor_scalar(
                out=xn_t[:],
                in0=x_t[:],
                scalar1=-1.0,
                scalar2=None,
                op0=AluOp.mult,
            )
            acc_u = persist.tile([128, SLABS], F32, tag="accu")
            acc_r = persist.tile([128, SLABS], F32, tag="accr")
            negone = persist.tile([128, 1], F32, tag="negone")
            nc.gpsimd.memset(negone[:], -1.0)

            # Each z0-group s writes two tile-pairs: cur (slab s, w0 taps) and
            # nxt (slab s+1, w1 taps). Each tile sees one contiguous PSUM
            # accumulation group; slab s's total = cur(s) + nxt from group
            # s-1, summed during evacuation.
            prev = None
            for s in range(SLABS):
                cur = psum.tile([128, 512], F32, tag="bank")
                nxt = psum.tile([128, 512], F32, tag="bank")
                nb = NBS[s]
                for gy in range(YHALVES):
                    for gx in range(XHALVES):
                        if True:
                            qq = gy * XHALVES + gx
                            base = offs_core[s * NQ + qq]
                            cr = cur[:, qq * 128 : (qq + 1) * 128]
                            nr = nxt[:, qq * 128 : (qq + 1) * 128]
                            for sb0, sbn in _sb_chunks(nb):
                                da = eab.tile([128, 16 * 128], BF16, tag="da")
                                eb = eab.tile([128, 16 * 128], BF16, tag="eb")
                                for j in range(sbn):
                                    c = base + sb0 + j
                                    nc.vector.tensor_scalar(
                                        out=da[:, j * 128 : (j + 1) * 128],
                                        in0=iota_t[:, gy * 128 : gy * 128 + 128],
                                        scalar1=y_t[:, c : c + 1],
                                        scalar2=None,
                                        op0=AluOp.subtract,
                                    )
                                    # |iota - xc| in one ACT op (bias = -xc)
                                    nc.scalar.activation(
                                        out=eb[:, j * 128 : (j + 1) * 128],
                                        in_=iota_t[:, gx * 128 : gx * 128 + 128],
                                        func=Act.Abs,
                                        bias=xn_t[:, c : c + 1],
                                        scale=1.0,
                                    )
                                ea = eab.tile([128, 16 * 128], BF16, tag="ea")
                                nc.scalar.activation(
                                    out=ea[:, : sbn * 128],
                                    in_=da[:, : sbn * 128],
                                    func=Act.Abs,
                                    bias=0.0,
                                    scale=1.0,
                                )
                                ta = tab.tile([128, 16 * 128], BF16, tag="ta")
                                tb = tab.tile([128, 16 * 128], BF16, tag="tb")
                                # negated tents min(|t|-1, 0); negations cancel
                                nc.vector.tensor_scalar(
                                    out=ta[:, : sbn * 128],
                                    in0=ea[:, : sbn * 128],
                                    scalar1=1.0,
                                    scalar2=0.0,
                                    op0=AluOp.subtract,
                                    op1=AluOp.min,
                                )
                                nc.vector.tensor_scalar(
                                    out=tb[:, : sbn * 128],
                                    in0=eb[:, : sbn * 128],
                                    scalar1=1.0,
                                    scalar2=0.0,
                                    op0=AluOp.subtract,
                                    op1=AluOp.min,
                                )
                                for j in range(sbn):
                                    c = base + sb0 + j
                                    first = sb0 + j == 0
                                    last = sb0 + j == nb - 1
                                    a0 = atile.tile([128, 128], BF16, tag="a0")
                                    nc.vector.tensor_scalar(
                                        out=a0[:],
                                        in0=ta[:, j * 128 : (j + 1) * 128],
                                        scalar1=w0_t[:, c : c + 1],
                                        scalar2=None,
                                        op0=AluOp.mult,
                                    )
                                    nc.tensor.matmul(
                                        cr,
                                        a0[:],
                                        tb[:, j * 128 : (j + 1) * 128],
                                        start=first,
                                        stop=last,
                                    )
                                    a1 = atile.tile([128, 128], BF16, tag="a1")
                                    nc.vector.tensor_scalar(
                                        out=a1[:],
                                        in0=ta[:, j * 128 : (j + 1) * 128],
                                        scalar1=w1_t[:, c : c + 1],
                                        scalar2=None,
                                        op0=AluOp.mult,
                                    )
                                    nc.tensor.matmul(
                                        nr,
                                        a1[:],
                                        tb[:, j * 128 : (j + 1) * 128],
                                        start=first,
                                        stop=last,
                                    )
                # evacuate slab s: d = cur(s) + prev-group nxt; then Huber
                d_sb = evac.tile([128, 512], BF16, tag="dsb")
                if prev is None:
                    nc.vector.tensor_copy(out=d_sb[:], in_=cur[:])
                else:
                    pp = evac.tile([128, 512], BF16, tag="pp")
                    nc.vector.tensor_copy(out=pp[:], in_=prev[:])
                    nc.vector.tensor_tensor(
                        out=d_sb[:], in0=cur[:], in1=pp[:], op=AluOp.add
                    )
                u = evac.tile([128, 512], BF16, tag="u")
                nc.vector.scalar_tensor_tensor(
                    out=u[:],
                    in0=d_sb[:],
                    scalar=-1.0,
                    in1=d_sb[:],
                    op0=AluOp.mult,
                    op1=AluOp.max,
                )
                r = evac.tile([128, 512], BF16, tag="r")
                nc.scalar.activation(
                    out=r[:], in_=u[:], func=Act.Relu, bias=negone[:], scale=1.0
                )
                squ = evac.tile([128, 512], BF16, tag="squ")
                nc.scalar.activation(
                    out=squ[:],
                    in_=u[:],
                    func=Act.Square,
                    accum_out=acc_u[:, s : s + 1],
                )
                sqr = evac.tile([128, 512], BF16, tag="sqr")
                nc.scalar.activation(
                    out=sqr[:],
                    in_=r[:],
                    func=Act.Square,
                    accum_out=acc_r[:, s : s + 1],
                )
                prev = nxt
            nc.sync.dma_start(out=OUTd[:, 0:SLABS], in_=acc_u[:])
            nc.sync.dma_start(out=OUTd[:, SLABS : 2 * SLABS], in_=acc_r[:])
    nc.compile()
    return nc


_PROGRAM_CACHE = {}


def _get_program():
    if "nc" not in _PROGRAM_CACHE:
        _PROGRAM_CACHE["nc"] = _build_program()
    return _PROGRAM_CACHE["nc"]


def _iota_input():
    return np.broadcast_to(
        np.arange(GRID, dtype=ml_dtypes.bfloat16)[None, :], (128, GRID)
    ).copy()


def kernel(registration_pred, registration_gt, coords, _trace=False):
    shards = _prepare_shards(registration_pred, registration_gt, coords)
    iota = _iota_input()
    nc = _get_program()
    in_maps = [
        {"Y": y, "X": x, "W0": w0, "W1": w1, "IOTA": iota}
        for (y, x, w0, w1) in shards
    ]
    try:
        res = run_bass_kernel_spmd(nc, in_maps, list(range(CORES)), trace=_trace)
    except Exception:
        # Transient device wedge (e.g. NRT_EXEC_UNIT_UNRECOVERABLE) has been
        # observed to fail a single run and recover on retry.
        res = run_bass_kernel_spmd(nc, in_maps, list(range(CORES)), trace=_trace)
    total = 0.0
    for r in res.results:
        out = r["OUT"].astype(np.float64)
        total += 0.5 * (out[:, :SLABS].sum() - out[:, SLABS:].sum())
    if _trace:
        kernel.last_exec_time_ns = res.exec_time_ns
        kernel.last_results = res
    return np.float32(total)



# revision 11
# speedup vs baseline: 2.6786x; 2.6786x over previous
"""Trainium2 Bass kernel for nn_DivrocLoss (trilinear splat histogram + Huber loss).

Strategy (8 NeuronCores, SPMD), v2 "interleaved chunk pipeline":
  - Spatial sharding over z: core c owns slabs [32c, 32c+32). Each (point,
    volume) pair is one record carrying window-local y/x coords and both
    z-tap weights (sign folded in). Records binned by (z-slab, y-window,
    x-window) with 64-wide y/x windows (16 bins per slab position); bin
    capacities are per-(pos, bin) maxima over cores, derived from the data.
  - On device, batches of 128 records are processed in chunks of C=32
    batches. All elementwise work is chunk-wide (no per-batch vector ops):
    tiles are stored interleaved (index k*cw + j for window coord k, batch
    j) so per-batch scalars become stride-0 broadcast reads:
      da  = iota - y           (DVE tensor_tensor, broadcast in1, 2x mode)
      ea  = |da|               (ACT Abs, flat)
      nty = min(ea - 1, 0)     (DVE tensor_scalar, 4x mode)  [negated tent]
      a0  = nty * w0           (DVE tensor_tensor, broadcast)
      a1  = nty * w1           (Pool/GpSimd tensor_tensor, broadcast)
      db/ab/ntx                (same, x side)
    Work is balanced across DVE / ACT / Pool engines; PE does 2 matmuls per
    batch (64-wide windows) with strided stationary/moving slices from the
    interleaved tiles.
  - Fused PSUM accumulation: slab s's PSUM tile receives group (s-1)'s
    w1-tap matmuls and group s's w0-tap matmuls in one accumulation group,
    so no cur+prev add is needed at evacuation. One [128, 512] PSUM bank
    holds all 16 bins of a slab (partition = (yh%2)*64+y, free =
    (yh//2)*256 + xh*64 + x).
  - Evacuation computes Huber partials via u=|d| (DVE), r=max(u-1,0) (DVE),
    and ACT Square with per-slab accumulators; host sums 0.5*(sum u^2 -
    sum r^2) over cores.
"""

import sys

sys.path.insert(0, "/opt/trn_rl_repo")

import numpy as np
import ml_dtypes

from concourse import bacc, bass, mybir
import concourse.tile as tile
from concourse.bass_utils import run_bass_kernel_spmd

GRID = 256
CORES = 8
SLABS = 32          # z slabs per core
WIN = 64            # y/x window width
NH = GRID // WIN    # 4 windows per axis
NQ = NH * NH        # 16 (yh, xh) bins per slab position
C = 32              # batches per chunk

F32 = mybir.dt.float32
BF16 = mybir.dt.bfloat16
F16 = mybir.dt.float16

_CACHE = {}


def _gen_records(coords, reg, sign):
    """Records for one volume: z-slab assignment with straddle split, y/x
    window assignment with boundary duplication. Mirrors the reference's f32
    unnormalization expression exactly."""
    p = coords + reg.astype(np.float32)
    g = ((p + np.float32(1.0)) * np.float32(GRID) - np.float32(1.0)) * np.float32(0.5)
    gx, gy, gz = g[:, 0], g[:, 1], g[:, 2]
    z0f = np.floor(gz)
    fz = (gz - z0f).astype(np.float32)
    z0 = z0f.astype(np.int64)
    w0 = (1.0 - fz) * sign
    w1 = fz * sign
    shift = z0 == -1          # only the z=0 tap is in grid
    w0 = np.where(shift, fz * sign, w0)
    w1 = np.where(shift, 0.0, w1)
    z0 = np.where(shift, 0, z0)
    w1 = np.where(z0 == GRID - 1, 0.0, w1)
    keep = (z0 >= 0) & (z0 <= GRID - 1)
    z0, gy, gx, w0, w1 = z0[keep], gy[keep], gx[keep], w0[keep], w1[keep]
    # split records whose z taps straddle a core boundary
    strad = ((z0 % SLABS) == SLABS - 1) & (z0 < GRID - 1)
    w1a = np.where(strad, 0.0, w1)
    recs = [
        (z0, gy, gx, w0, w1a),
        (z0[strad] + 1, gy[strad], gx[strad], w1[strad],
         np.zeros(int(strad.sum()), np.float32)),
    ]
    out = []
    for zz, gyv, gxv, rw0, rw1 in recs:
        y0 = np.floor(gyv).astype(np.int64)
        x0 = np.floor(gxv).astype(np.int64)
        yh = np.clip(y0 // WIN, 0, NH - 1)
        xh = np.clip(x0 // WIN, 0, NH - 1)
        dupy = (y0 % WIN == WIN - 1) & (y0 >= 0) & (y0 < GRID - 1)
        dupx = (x0 % WIN == WIN - 1) & (x0 >= 0) & (x0 < GRID - 1)
        dupyx = dupy & dupx
        for sel, byh, bxh in (
            (slice(None), yh, xh),
            (dupy, yh[dupy] + 1, xh[dupy]),
            (dupx, yh[dupx], xh[dupx] + 1),
            (dupyx, yh[dupyx] + 1, xh[dupyx] + 1),
        ):
            out.append((zz[sel], gyv[sel], gxv[sel], rw0[sel], rw1[sel],
                        byh, bxh))
    return out


def _prepare(registration_pred, registration_gt, coords):
    """Build per-core field tiles + the cap table."""
    coords = coords.astype(np.float32)
    parts = []
    for reg, sign in ((registration_pred, np.float32(1.0)),
                      (registration_gt, np.float32(-1.0))):
        parts.extend(_gen_records(coords, reg, sign))
    Z = np.concatenate([p[0] for p in parts])
    GY = np.concatenate([p[1] for p in parts])
    GX = np.concatenate([p[2] for p in parts])
    W0 = np.concatenate([p[3] for p in parts])
    W1 = np.concatenate([p[4] for p in parts])
    YH = np.concatenate([np.broadcast_to(p[5], p[0].shape) for p in parts])
    XH = np.concatenate([np.broadcast_to(p[6], p[0].shape) for p in parts])

    core = Z // SLABS
    pos = Z % SLABS
    q = YH * NH + XH
    gbin = (core * SLABS + pos) * NQ + q
    nbins = GRID * NQ
    counts = np.bincount(gbin, minlength=nbins)
    caps = np.maximum(
        1,
        np.ceil(counts.reshape(CORES, SLABS, NQ).max(axis=0) / 128).astype(np.int64),
    )  # [SLABS, NQ]

    # column offsets per (pos, q), shared by all cores
    flat_caps = caps.reshape(-1)
    col_off = np.zeros(SLABS * NQ, dtype=np.int64)
    np.cumsum(flat_caps[:-1], out=col_off[1:])
    TOT = int(flat_caps.sum())

    order = np.argsort(gbin, kind="stable")
    Z, GY, GX, W0, W1, YH, XH = (a[order] for a in (Z, GY, GX, W0, W1, YH, XH))
    gbin = gbin[order]
    starts = np.zeros(nbins + 1, dtype=np.int64)
    np.cumsum(counts, out=starts[1:])
    rank = np.arange(len(gbin), dtype=np.int64) - starts[gbin]
    core_of = gbin // (SLABS * NQ)
    local_bin = gbin % (SLABS * NQ)
    col = col_off[local_bin] + rank // 128
    part = rank % 128
    dest = (core_of * TOT + col) * 128 + part

    yl = (GY - (YH * WIN + np.float32(WIN / 2 - 0.5))).astype(np.float16)
    xl = (GX - (XH * WIN + np.float32(WIN / 2 - 0.5))).astype(np.float16)

    def field(vals, dtype):
        flat = np.zeros(CORES * TOT * 128, dtype=dtype)
        flat[dest] = vals
        out = []
        for c in range(CORES):
            block = flat[c * TOT * 128:(c + 1) * TOT * 128]
            out.append(np.ascontiguousarray(block.reshape(TOT, 128).T))
        return out

    shards = list(zip(
        field(yl, np.float16),
        field(xl, np.float16),
        field(W0.astype(ml_dtypes.bfloat16), ml_dtypes.bfloat16),
        field(W1.astype(ml_dtypes.bfloat16), ml_dtypes.bfloat16),
    ))
    return shards, caps, col_off, TOT


def _iota_interleaved():
    k = np.arange(WIN, dtype=np.float32) - np.float32(WIN / 2 - 0.5)
    row = np.repeat(k, C)  # iota_i[k*C + j] = k - 31.5
    return np.broadcast_to(row.astype(np.float16)[None, :], (128, WIN * C)).copy()


def _build_program(caps, TOT, dbg=False):
    nc = bacc.Bacc("TRN2", target_bir_lowering=False, debug=False,
                   num_devices=CORES)
    YLd = nc.declare_dram_parameter("YL", [128, TOT], F16, isOutput=False)
    XLd = nc.declare_dram_parameter("XL", [128, TOT], F16, isOutput=False)
    W0d = nc.declare_dram_parameter("W0", [128, TOT], BF16, isOutput=False)
    W1d = nc.declare_dram_parameter("W1", [128, TOT], BF16, isOutput=False)
    IOd = nc.declare_dram_parameter("IOTA", [128, WIN * C], F16, isOutput=False)
    OUTd = nc.declare_dram_parameter("OUT", [128, 2 * SLABS], F32, isOutput=True)
    DBGd = (nc.declare_dram_parameter("DBG", [128, 512 * SLABS], F32,
                                      isOutput=True) if dbg else None)

    AluOp = mybir.AluOpType
    Act = mybir.ActivationFunctionType

    # per-column metadata: (pos, q, idx, cap)
    bmeta = []
    for s in range(SLABS):
        for qq in range(NQ):
            cap = int(caps[s, qq])
            for i in range(cap):
                bmeta.append((s, qq, i, cap))
    assert len(bmeta) == TOT

    def binslice(t, qq):
        yh, xh = qq // NH, qq % NH
        p0 = (yh % 2) * WIN
        f0 = (yh // 2) * (NH * WIN) + xh * WIN
        return t[p0:p0 + WIN, f0:f0 + WIN]

    with tile.TileContext(nc) as tc:
        with (
            tc.tile_pool(name="persist", bufs=1) as persist,
            tc.tile_pool(name="chunkp", bufs=3) as chunkp,
            tc.tile_pool(name="evac", bufs=2) as evac,
            tc.tile_pool(name="psum", bufs=8, space="PSUM") as psum,
        ):
            yl_t = persist.tile([128, TOT], F16, tag="yl")
            nc.sync.dma_start(out=yl_t[:], in_=YLd[:])
            xl_t = persist.tile([128, TOT], F16, tag="xl")
            nc.sync.dma_start(out=xl_t[:], in_=XLd[:])
            w0_t = persist.tile([128, TOT], BF16, tag="w0")
            nc.sync.dma_start(out=w0_t[:], in_=W0d[:])
            w1_t = persist.tile([128, TOT], BF16, tag="w1")
            nc.sync.dma_start(out=w1_t[:], in_=W1d[:])
            iota_t = persist.tile([128, WIN * C], F16, tag="iota")
            nc.sync.dma_start(out=iota_t[:], in_=IOd[:])
            acc_u = persist.tile([128, SLABS], F32, tag="accu")
            acc_r = persist.tile([128, SLABS], F32, tag="accr")
            zero_t = persist.tile([128, 512], BF16, tag="zero")
            nc.gpsimd.memset(zero_t[:], 0.0)

            ptiles = {}

            def get_ptile(s):
                if s not in ptiles:
                    t = psum.tile([128, NQ * WIN * WIN // 128], F32,
                                  tag="bank", name=f"bank{s}")
                    ptiles[s] = t
                    # full-bank accumulation-group start: pends + zeroes the
                    # whole bank so per-bin matmuls can all accumulate
                    nc.tensor.matmul(t[:], zero_t[:, 0:128], zero_t[:],
                                     start=True, stop=False)
                return ptiles[s]

            def evacuate(s):
                d = ptiles.pop(s)
                # full-bank group stop (accumulates zero)
                nc.tensor.matmul(d[:], zero_t[:, 0:128], zero_t[:],
                                 start=False, stop=True)
                if dbg:
                    dc = evac.tile([128, 512], F32, tag="dbgc")
                    nc.vector.tensor_copy(out=dc[:], in_=d[:])
                    nc.sync.dma_start(out=DBGd[:, s * 512:(s + 1) * 512],
                                      in_=dc[:])
                u = evac.tile([128, 512], BF16, tag="u")
                nc.scalar.activation(out=u[:], in_=d[:], func=Act.Abs)
                squ = evac.tile([128, 512], BF16, tag="squ")
                nc.scalar.activation(
                    out=squ[:], in_=u[:], func=Act.Square,
                    accum_out=acc_u[:, s:s + 1],
                )
                r = evac.tile([128, 512], BF16, tag="r")
                nc.vector.tensor_scalar(
                    out=r[:], in0=u[:], scalar1=1.0, scalar2=0.0,
                    op0=AluOp.subtract, op1=AluOp.max,
                )
                sqr = evac.tile([128, 512], BF16, tag="sqr")
                nc.scalar.activation(
                    out=sqr[:], in_=r[:], func=Act.Square,
                    accum_out=acc_r[:, s:s + 1],
                )

            for cc in range(0, TOT, C):
                cw = min(C, TOT - cc)
                n = cw * WIN

                def iv(t, width=None):
                    """interleaved 3D view [128, WIN, cw] of a chunk tile"""
                    w = width or cw
                    return t[:, :WIN * w].rearrange("p (k j) -> p k j", j=w)

                iota_v = iota_t[:].rearrange("p (k j) -> p k j", j=C)[:, :, :cw]

                def bc(t):
                    return t[:, cc:cc + cw].unsqueeze(1).broadcast_to(
                        (128, WIN, cw))

                da = chunkp.tile([128, WIN * C], BF16, tag="da")
                nc.vector.tensor_tensor(out=iv(da), in0=iota_v, in1=bc(yl_t),
                                        op=AluOp.subtract)
                ea = chunkp.tile([128, WIN * C], BF16, tag="ea")
                nc.scalar.activation(out=ea[:, :n], in_=da[:, :n], func=Act.Abs)
                nty = chunkp.tile([128, WIN * C], BF16, tag="nty")
                nc.vector.tensor_scalar(out=nty[:, :n], in0=ea[:, :n],
                                        scalar1=1.0, scalar2=0.0,
                                        op0=AluOp.subtract, op1=AluOp.min)
                a0 = chunkp.tile([128, WIN * C], BF16, tag="a0")
                nc.vector.tensor_tensor(out=iv(a0), in0=iv(nty), in1=bc(w0_t),
                                        op=AluOp.mult)
                a1 = chunkp.tile([128, WIN * C], BF16, tag="a1")
                nc.gpsimd.tensor_tensor(out=iv(a1), in0=iv(nty), in1=bc(w1_t),
                                        op=AluOp.mult)
                db = chunkp.tile([128, WIN * C], BF16, tag="db")
                nc.vector.tensor_tensor(out=iv(db), in0=iota_v, in1=bc(xl_t),
                                        op=AluOp.subtract)
                ab = chunkp.tile([128, WIN * C], BF16, tag="ab")
                nc.scalar.activation(out=ab[:, :n], in_=db[:, :n], func=Act.Abs)
                ntx = chunkp.tile([128, WIN * C], BF16, tag="ntx")
                nc.vector.tensor_scalar(out=ntx[:, :n], in0=ab[:, :n],
                                        scalar1=1.0, scalar2=0.0,
                                        op0=AluOp.subtract, op1=AluOp.min)

                a0v, a1v, ntxv = iv(a0), iv(a1), iv(ntx)
                for j in range(cw):
                    s, qq, idx, cap = bmeta[cc + j]
                    mov = ntxv[:, :, j]
                    # w0 tap -> slab s (group: prior nxt then these cur)
                    nc.tensor.matmul(
                        binslice(get_ptile(s), qq), a0v[:, :, j], mov,
                        start=False, stop=False,
                    )
                    # w1 tap -> slab s+1 (skipped for pos 31: w1 == 0 there)
                    if s < SLABS - 1:
                        nc.tensor.matmul(
                            binslice(get_ptile(s + 1), qq), a1v[:, :, j], mov,
                            start=False, stop=False,
                        )
                    if idx == cap - 1 and qq == NQ - 1:
                        evacuate(s)

            nc.sync.dma_start(out=OUTd[:, 0:SLABS], in_=acc_u[:])
            nc.sync.dma_start(out=OUTd[:, SLABS:2 * SLABS], in_=acc_r[:])
    nc.compile()
    return nc


def _get_program():
    return _CACHE["nc"]


def kernel(registration_pred, registration_gt, coords, _trace=False):
    shards, caps, col_off, TOT = _prepare(registration_pred, registration_gt,
                                          coords)
    key = (TOT, caps.tobytes())
    if _CACHE.get("key") != key:
        _CACHE["nc"] = _build_program(caps, TOT)
        _CACHE["key"] = key
    nc = _CACHE["nc"]
    iota = _iota_interleaved()
    in_maps = [
        {"YL": yl, "XL": xl, "W0": w0, "W1": w1, "IOTA": iota}
        for (yl, xl, w0, w1) in shards
    ]
    try:
        res = run_bass_kernel_spmd(nc, in_maps, list(range(CORES)),
                                   trace=_trace)
    except Exception:
        res = run_bass_kernel_spmd(nc, in_maps, list(range(CORES)),
                                   trace=_trace)
    total = 0.0
    for r in res.results:
        out = r["OUT"].astype(np.float64)
        total += 0.5 * (out[:, :SLABS].sum() - out[:, SLABS:].sum())
    if _trace:
        kernel.last_exec_time_ns = res.exec_time_ns
        kernel.last_results = res
    return np.float32(total)


# revision 15
# speedup vs baseline: 3.9739x; 1.4836x over previous
"""Trainium2 Bass kernel for nn_DivrocLoss (trilinear splat histogram + Huber loss).

Strategy (8 NeuronCores, SPMD), v2 "interleaved chunk pipeline":
  - Spatial sharding over z: core c owns slabs [32c, 32c+32). Each (point,
    volume) pair is one record carrying window-local y/x coords and both
    z-tap weights (sign folded in). Records binned by (z-slab, y-window,
    x-window) with 64-wide y/x windows (16 bins per slab position); bin
    capacities are per-(pos, bin) maxima over cores, derived from the data.
  - On device, batches of 128 records are processed in chunks of C=32
    batches. All elementwise work is chunk-wide (no per-batch vector ops):
    tiles are stored interleaved (index k*cw + j for window coord k, batch
    j) so per-batch scalars become stride-0 broadcast reads:
      da  = iota - y           (DVE tensor_tensor, broadcast in1, 2x mode)
      ea  = |da|               (ACT Abs, flat)
      nty = min(ea - 1, 0)     (DVE tensor_scalar, 4x mode)  [negated tent]
      a0  = nty * w0           (DVE tensor_tensor, broadcast)
      a1  = nty * w1           (Pool/GpSimd tensor_tensor, broadcast)
      db/ab/ntx                (same, x side)
    Work is balanced across DVE / ACT / Pool engines; PE does 2 matmuls per
    batch (64-wide windows) with strided stationary/moving slices from the
    interleaved tiles.
  - Fused PSUM accumulation: slab s's PSUM tile receives group (s-1)'s
    w1-tap matmuls and group s's w0-tap matmuls in one accumulation group,
    so no cur+prev add is needed at evacuation. One [128, 512] PSUM bank
    holds all 16 bins of a slab (partition = (yh%2)*64+y, free =
    (yh//2)*256 + xh*64 + x).
  - Evacuation computes Huber partials via u=|d| (DVE), r=max(u-1,0) (DVE),
    and ACT Square with per-slab accumulators; host sums 0.5*(sum u^2 -
    sum r^2) over cores.
"""

import sys

sys.path.insert(0, "/opt/trn_rl_repo")

import numpy as np
import ml_dtypes

from concourse import bacc, bass, mybir
import concourse.tile as tile
from concourse.bass_utils import run_bass_kernel_spmd

GRID = 256
CORES = 8
SLABS = 32          # z slabs per core
WIN = 32            # y/x window width
NH = GRID // WIN    # windows per axis
NQ = NH * NH        # (yh, xh) bins per slab position
PPW = 128 // WIN    # windows stacked along PSUM partitions
C = 32              # batches per chunk

F32 = mybir.dt.float32
BF16 = mybir.dt.bfloat16
F16 = mybir.dt.float16

_CACHE = {}


def _gen_records(coords, reg, sign):
    """Records for one volume: z-slab assignment with straddle split, y/x
    window assignment with boundary duplication. Mirrors the reference's f32
    unnormalization expression exactly."""
    p = coords + reg.astype(np.float32)
    g = ((p + np.float32(1.0)) * np.float32(GRID) - np.float32(1.0)) * np.float32(0.5)
    gx, gy, gz = g[:, 0], g[:, 1], g[:, 2]
    z0f = np.floor(gz)
    fz = (gz - z0f).astype(np.float32)
    z0 = z0f.astype(np.int64)
    w0 = (1.0 - fz) * sign
    w1 = fz * sign
    shift = z0 == -1          # only the z=0 tap is in grid
    w0 = np.where(shift, fz * sign, w0)
    w1 = np.where(shift, 0.0, w1)
    z0 = np.where(shift, 0, z0)
    w1 = np.where(z0 == GRID - 1, 0.0, w1)
    keep = (z0 >= 0) & (z0 <= GRID - 1)
    z0, gy, gx, w0, w1 = z0[keep], gy[keep], gx[keep], w0[keep], w1[keep]
    # split records whose z taps straddle a core boundary
    strad = ((z0 % SLABS) == SLABS - 1) & (z0 < GRID - 1)
    w1a = np.where(strad, 0.0, w1)
    recs = [
        (z0, gy, gx, w0, w1a),
        (z0[strad] + 1, gy[strad], gx[strad], w1[strad],
         np.zeros(int(strad.sum()), np.float32)),
    ]
    out = []
    for zz, gyv, gxv, rw0, rw1 in recs:
        y0 = np.floor(gyv).astype(np.int64)
        x0 = np.floor(gxv).astype(np.int64)
        yh = np.clip(y0 // WIN, 0, NH - 1)
        xh = np.clip(x0 // WIN, 0, NH - 1)
        dupy = (y0 % WIN == WIN - 1) & (y0 >= 0) & (y0 < GRID - 1)
        dupx = (x0 % WIN == WIN - 1) & (x0 >= 0) & (x0 < GRID - 1)
        dupyx = dupy & dupx
        for sel, byh, bxh in (
            (slice(None), yh, xh),
            (dupy, yh[dupy] + 1, xh[dupy]),
            (dupx, yh[dupx], xh[dupx] + 1),
            (dupyx, yh[dupyx] + 1, xh[dupyx] + 1),
        ):
            out.append((zz[sel], gyv[sel], gxv[sel], rw0[sel], rw1[sel],
                        byh, bxh))
    return out


def _prepare(registration_pred, registration_gt, coords):
    """Build per-core field tiles + the cap table."""
    coords = coords.astype(np.float32)
    parts = []
    for reg, sign in ((registration_pred, np.float32(1.0)),
                      (registration_gt, np.float32(-1.0))):
        parts.extend(_gen_records(coords, reg, sign))
    Z = np.concatenate([p[0] for p in parts])
    GY = np.concatenate([p[1] for p in parts])
    GX = np.concatenate([p[2] for p in parts])
    W0 = np.concatenate([p[3] for p in parts])
    W1 = np.concatenate([p[4] for p in parts])
    YH = np.concatenate([np.broadcast_to(p[5], p[0].shape) for p in parts])
    XH = np.concatenate([np.broadcast_to(p[6], p[0].shape) for p in parts])

    core = Z // SLABS
    pos = Z % SLABS
    q = YH * NH + XH
    gbin = (core * SLABS + pos) * NQ + q
    nbins = GRID * NQ
    counts = np.bincount(gbin, minlength=nbins)
    caps = np.maximum(
        1,
        np.ceil(counts.reshape(CORES, SLABS, NQ).max(axis=0) / 128).astype(np.int64),
    )  # [SLABS, NQ]

    # column offsets per (pos, q), shared by all cores
    flat_caps = caps.reshape(-1)
    col_off = np.zeros(SLABS * NQ, dtype=np.int64)
    np.cumsum(flat_caps[:-1], out=col_off[1:])
    TOT = int(flat_caps.sum())

    order = np.argsort(gbin, kind="stable")
    Z, GY, GX, W0, W1, YH, XH = (a[order] for a in (Z, GY, GX, W0, W1, YH, XH))
    gbin = gbin[order]
    starts = np.zeros(nbins + 1, dtype=np.int64)
    np.cumsum(counts, out=starts[1:])
    rank = np.arange(len(gbin), dtype=np.int64) - starts[gbin]
    core_of = gbin // (SLABS * NQ)
    local_bin = gbin % (SLABS * NQ)
    col = col_off[local_bin] + rank // 128
    part = rank % 128
    dest = (core_of * TOT + col) * 128 + part

    yl = (GY - (YH * WIN + np.float32(WIN / 2 - 0.5))).astype(np.float16)
    xl = (GX - (XH * WIN + np.float32(WIN / 2 - 0.5))).astype(np.float16)

    def field(vals, dtype):
        flat = np.zeros(CORES * TOT * 128, dtype=dtype)
        flat[dest] = vals
        out = []
        for c in range(CORES):
            block = flat[c * TOT * 128:(c + 1) * TOT * 128]
            out.append(np.ascontiguousarray(block.reshape(TOT, 128).T))
        return out

    shards = list(zip(
        field(yl, np.float16),
        field(xl, np.float16),
        field(W0.astype(ml_dtypes.bfloat16), ml_dtypes.bfloat16),
        field(W1.astype(ml_dtypes.bfloat16), ml_dtypes.bfloat16),
    ))
    return shards, caps, col_off, TOT


def _iota_interleaved():
    k = np.arange(WIN, dtype=np.float32) - np.float32(WIN / 2 - 0.5)
    row = np.repeat(k, C)  # iota_i[k*C + j] = k - 31.5
    return np.broadcast_to(row.astype(np.float16)[None, :], (128, WIN * C)).copy()


def _build_program(caps, TOT, dbg=False):
    nc = bacc.Bacc("TRN2", target_bir_lowering=False, debug=False,
                   num_devices=CORES)
    YLd = nc.declare_dram_parameter("YL", [128, TOT], F16, isOutput=False)
    XLd = nc.declare_dram_parameter("XL", [128, TOT], F16, isOutput=False)
    W0d = nc.declare_dram_parameter("W0", [128, TOT], BF16, isOutput=False)
    W1d = nc.declare_dram_parameter("W1", [128, TOT], BF16, isOutput=False)
    IOd = nc.declare_dram_parameter("IOTA", [128, WIN * C], F16, isOutput=False)
    OUTd = nc.declare_dram_parameter("OUT", [128, 2 * SLABS], F32, isOutput=True)
    DBGd = (nc.declare_dram_parameter("DBG", [128, 512 * SLABS], F32,
                                      isOutput=True) if dbg else None)

    AluOp = mybir.AluOpType
    Act = mybir.ActivationFunctionType

    # per-column metadata: (pos, q, idx, cap)
    bmeta = []
    for s in range(SLABS):
        for qq in range(NQ):
            cap = int(caps[s, qq])
            for i in range(cap):
                bmeta.append((s, qq, i, cap))
    assert len(bmeta) == TOT

    def binslice(t, qq):
        yh, xh = qq // NH, qq % NH
        p0 = (yh % PPW) * WIN
        f0 = (yh // PPW) * (NH * WIN) + xh * WIN
        return t[p0:p0 + WIN, f0:f0 + WIN], p0

    with tile.TileContext(nc) as tc:
        with (
            tc.tile_pool(name="persist", bufs=1) as persist,
            tc.tile_pool(name="chunkp", bufs=3) as chunkp,
            tc.tile_pool(name="evac", bufs=2) as evac,
            tc.tile_pool(name="psum", bufs=8, space="PSUM") as psum,
        ):
            yl_t = persist.tile([128, TOT], F16, tag="yl")
            nc.sync.dma_start(out=yl_t[:], in_=YLd[:])
            xl_t = persist.tile([128, TOT], F16, tag="xl")
            nc.sync.dma_start(out=xl_t[:], in_=XLd[:])
            w0_t = persist.tile([128, TOT], BF16, tag="w0")
            nc.sync.dma_start(out=w0_t[:], in_=W0d[:])
            w1_t = persist.tile([128, TOT], BF16, tag="w1")
            nc.sync.dma_start(out=w1_t[:], in_=W1d[:])
            iota_t = persist.tile([128, WIN * C], F16, tag="iota")
            nc.sync.dma_start(out=iota_t[:], in_=IOd[:])
            acc_u = persist.tile([128, SLABS], F32, tag="accu")
            acc_r = persist.tile([128, SLABS], F32, tag="accr")
            zero_t = persist.tile([128, 512], BF16, tag="zero")
            nc.gpsimd.memset(zero_t[:], 0.0)

            ptiles = {}

            def get_ptile(s):
                if s not in ptiles:
                    t = psum.tile([128, NQ * WIN * WIN // 128], F32,
                                  tag="bank", name=f"bank{s}")
                    ptiles[s] = t
                    # full-bank accumulation-group start: pends + zeroes the
                    # whole bank so per-bin matmuls can all accumulate
                    nc.tensor.matmul(t[:], zero_t[:, 0:128], zero_t[:],
                                     start=True, stop=False)
                return ptiles[s]

            def evacuate(s):
                d = ptiles.pop(s)
                # full-bank group stop (accumulates zero)
                nc.tensor.matmul(d[:], zero_t[:, 0:128], zero_t[:],
                                 start=False, stop=True)
                if dbg:
                    dc = evac.tile([128, 512], F32, tag="dbgc")
                    nc.vector.tensor_copy(out=dc[:], in_=d[:])
                    nc.sync.dma_start(out=DBGd[:, s * 512:(s + 1) * 512],
                                      in_=dc[:])
                u = evac.tile([128, 512], BF16, tag="u")
                nc.scalar.activation(out=u[:], in_=d[:], func=Act.Abs)
                squ = evac.tile([128, 512], BF16, tag="squ")
                nc.scalar.activation(
                    out=squ[:], in_=u[:], func=Act.Square,
                    accum_out=acc_u[:, s:s + 1],
                )
                r = evac.tile([128, 512], BF16, tag="r")
                nc.vector.tensor_scalar(
                    out=r[:], in0=u[:], scalar1=1.0, scalar2=0.0,
                    op0=AluOp.subtract, op1=AluOp.max,
                )
                sqr = evac.tile([128, 512], BF16, tag="sqr")
                nc.scalar.activation(
                    out=sqr[:], in_=r[:], func=Act.Square,
                    accum_out=acc_r[:, s:s + 1],
                )

            for cc in range(0, TOT, C):
                cw = min(C, TOT - cc)
                n = cw * WIN

                def iv(t, width=None):
                    """interleaved 3D view [128, WIN, cw] of a chunk tile"""
                    w = width or cw
                    return t[:, :WIN * w].rearrange("p (k j) -> p k j", j=w)

                iota_v = iota_t[:].rearrange("p (k j) -> p k j", j=C)[:, :, :cw]

                def bc(t):
                    return t[:, cc:cc + cw].unsqueeze(1).broadcast_to(
                        (128, WIN, cw))

                da = chunkp.tile([128, WIN * C], BF16, tag="da")
                nc.vector.tensor_tensor(out=iv(da), in0=iota_v, in1=bc(yl_t),
                                        op=AluOp.subtract)
                ea = chunkp.tile([128, WIN * C], BF16, tag="ea")
                nc.scalar.activation(out=ea[:, :n], in_=da[:, :n], func=Act.Abs)
                nty = chunkp.tile([128, WIN * C], BF16, tag="nty")
                nc.vector.tensor_scalar(out=nty[:, :n], in0=ea[:, :n],
                                        scalar1=1.0, scalar2=0.0,
                                        op0=AluOp.subtract, op1=AluOp.min)
                a0 = chunkp.tile([128, WIN * C], BF16, tag="a0")
                nc.vector.tensor_tensor(out=iv(a0), in0=iv(nty), in1=bc(w0_t),
                                        op=AluOp.mult)
                a1 = chunkp.tile([128, WIN * C], BF16, tag="a1")
                nc.gpsimd.tensor_tensor(out=iv(a1), in0=iv(nty), in1=bc(w1_t),
                                        op=AluOp.mult)
                db = chunkp.tile([128, WIN * C], BF16, tag="db")
                nc.vector.tensor_tensor(out=iv(db), in0=iota_v, in1=bc(xl_t),
                                        op=AluOp.subtract)
                ab = chunkp.tile([128, WIN * C], BF16, tag="ab")
                nc.scalar.activation(out=ab[:, :n], in_=db[:, :n], func=Act.Abs)
                ntx = chunkp.tile([128, WIN * C], BF16, tag="ntx")
                nc.vector.tensor_scalar(out=ntx[:, :n], in0=ab[:, :n],
                                        scalar1=1.0, scalar2=0.0,
                                        op0=AluOp.subtract, op1=AluOp.min)

                a0v, a1v, ntxv = iv(a0), iv(a1), iv(ntx)
                for j in range(cw):
                    s, qq, idx, cap = bmeta[cc + j]
                    mov = ntxv[:, :, j]
                    # w0 tap -> slab s (group: prior nxt then these cur)
                    outc, p0 = binslice(get_ptile(s), qq)
                    nc.tensor.matmul(
                        outc, a0v[:, :, j], mov,
                        start=False, stop=False, tile_position=(0, p0),
                    )
                    # w1 tap -> slab s+1 (skipped for pos 31: w1 == 0 there)
                    if s < SLABS - 1:
                        outn, p0n = binslice(get_ptile(s + 1), qq)
                        nc.tensor.matmul(
                            outn, a1v[:, :, j], mov,
                            start=False, stop=False, tile_position=(0, p0n),
                        )
                    if idx == cap - 1 and qq == NQ - 1:
                        evacuate(s)

            nc.sync.dma_start(out=OUTd[:, 0:SLABS], in_=acc_u[:])
            nc.sync.dma_start(out=OUTd[:, SLABS:2 * SLABS], in_=acc_r[:])
    nc.compile()
    return nc


def _get_program():
    return _CACHE["nc"]


def kernel(registration_pred, registration_gt, coords, _trace=False):
    shards, caps, col_off, TOT = _prepare(registration_pred, registration_gt,
                                          coords)
    key = (TOT, caps.tobytes())
    if _CACHE.get("key") != key:
        _CACHE["nc"] = _build_program(caps, TOT)
        _CACHE["key"] = key
    nc = _CACHE["nc"]
    iota = _iota_interleaved()
    in_maps = [
        {"YL": yl, "XL": xl, "W0": w0, "W1": w1, "IOTA": iota}
        for (yl, xl, w0, w1) in shards
    ]
    try:
        res = run_bass_kernel_spmd(nc, in_maps, list(range(CORES)),
                                   trace=_trace)
    except Exception:
        res = run_bass_kernel_spmd(nc, in_maps, list(range(CORES)),
                                   trace=_trace)
    total = 0.0
    for r in res.results:
        out = r["OUT"].astype(np.float64)
        total += 0.5 * (out[:, :SLABS].sum() - out[:, SLABS:].sum())
    if _trace:
        kernel.last_exec_time_ns = res.exec_time_ns
        kernel.last_results = res
    return np.float32(total)


# revision 18
# speedup vs baseline: 4.1803x; 1.0520x over previous
"""Trainium2 Bass kernel for nn_DivrocLoss (trilinear splat histogram + Huber loss).

Strategy (8 NeuronCores, SPMD), v2 "interleaved chunk pipeline":
  - Spatial sharding over z: core c owns slabs [32c, 32c+32). Each (point,
    volume) pair is one record carrying window-local y/x coords and both
    z-tap weights (sign folded in). Records binned by (z-slab, y-window,
    x-window) with 64-wide y/x windows (16 bins per slab position); bin
    capacities are per-(pos, bin) maxima over cores, derived from the data.
  - On device, batches of 128 records are processed in chunks of C=32
    batches. All elementwise work is chunk-wide (no per-batch vector ops):
    tiles are stored interleaved (index k*cw + j for window coord k, batch
    j) so per-batch scalars become stride-0 broadcast reads:
      da  = iota - y           (DVE tensor_tensor, broadcast in1, 2x mode)
      ea  = |da|               (ACT Abs, flat)
      nty = min(ea - 1, 0)     (DVE tensor_scalar, 4x mode)  [negated tent]
      a0  = nty * w0           (DVE tensor_tensor, broadcast)
      a1  = nty * w1           (Pool/GpSimd tensor_tensor, broadcast)
      db/ab/ntx                (same, x side)
    Work is balanced across DVE / ACT / Pool engines; PE does 2 matmuls per
    batch (64-wide windows) with strided stationary/moving slices from the
    interleaved tiles.
  - Fused PSUM accumulation: slab s's PSUM tile receives group (s-1)'s
    w1-tap matmuls and group s's w0-tap matmuls in one accumulation group,
    so no cur+prev add is needed at evacuation. One [128, 512] PSUM bank
    holds all 16 bins of a slab (partition = (yh%2)*64+y, free =
    (yh//2)*256 + xh*64 + x).
  - Evacuation computes Huber partials via u=|d| (DVE), r=max(u-1,0) (DVE),
    and ACT Square with per-slab accumulators; host sums 0.5*(sum u^2 -
    sum r^2) over cores.
"""

import sys

sys.path.insert(0, "/opt/trn_rl_repo")

import numpy as np
import ml_dtypes

from concourse import bacc, bass, mybir
import concourse.tile as tile
from concourse.bass_utils import run_bass_kernel_spmd

GRID = 256
CORES = 8
SLABS = 32          # z slabs per core
WIN = 32            # y/x window width
NH = GRID // WIN    # windows per axis
NQ = NH * NH        # (yh, xh) bins per slab position
PPW = 128 // WIN    # windows stacked along PSUM partitions
C = 64              # batches per chunk

F32 = mybir.dt.float32
BF16 = mybir.dt.bfloat16
F16 = mybir.dt.float16

_CACHE = {}


def _gen_records(coords, reg, sign):
    """Records for one volume: z-slab assignment with straddle split, y/x
    window assignment with boundary duplication. Mirrors the reference's f32
    unnormalization expression exactly."""
    p = coords + reg.astype(np.float32)
    g = ((p + np.float32(1.0)) * np.float32(GRID) - np.float32(1.0)) * np.float32(0.5)
    gx, gy, gz = g[:, 0], g[:, 1], g[:, 2]
    z0f = np.floor(gz)
    fz = (gz - z0f).astype(np.float32)
    z0 = z0f.astype(np.int64)
    w0 = (1.0 - fz) * sign
    w1 = fz * sign
    shift = z0 == -1          # only the z=0 tap is in grid
    w0 = np.where(shift, fz * sign, w0)
    w1 = np.where(shift, 0.0, w1)
    z0 = np.where(shift, 0, z0)
    w1 = np.where(z0 == GRID - 1, 0.0, w1)
    keep = (z0 >= 0) & (z0 <= GRID - 1)
    z0, gy, gx, w0, w1 = z0[keep], gy[keep], gx[keep], w0[keep], w1[keep]
    # split records whose z taps straddle a core boundary
    strad = ((z0 % SLABS) == SLABS - 1) & (z0 < GRID - 1)
    w1a = np.where(strad, 0.0, w1)
    recs = [
        (z0, gy, gx, w0, w1a),
        (z0[strad] + 1, gy[strad], gx[strad], w1[strad],
         np.zeros(int(strad.sum()), np.float32)),
    ]
    out = []
    for zz, gyv, gxv, rw0, rw1 in recs:
        y0 = np.floor(gyv).astype(np.int64)
        x0 = np.floor(gxv).astype(np.int64)
        yh = np.clip(y0 // WIN, 0, NH - 1)
        xh = np.clip(x0 // WIN, 0, NH - 1)
        dupy = (y0 % WIN == WIN - 1) & (y0 >= 0) & (y0 < GRID - 1)
        dupx = (x0 % WIN == WIN - 1) & (x0 >= 0) & (x0 < GRID - 1)
        dupyx = dupy & dupx
        for sel, byh, bxh in (
            (slice(None), yh, xh),
            (dupy, yh[dupy] + 1, xh[dupy]),
            (dupx, yh[dupx], xh[dupx] + 1),
            (dupyx, yh[dupyx] + 1, xh[dupyx] + 1),
        ):
            out.append((zz[sel], gyv[sel], gxv[sel], rw0[sel], rw1[sel],
                        byh, bxh))
    return out


def _prepare(registration_pred, registration_gt, coords):
    """Build per-core field tiles + the cap table."""
    coords = coords.astype(np.float32)
    parts = []
    for reg, sign in ((registration_pred, np.float32(1.0)),
                      (registration_gt, np.float32(-1.0))):
        parts.extend(_gen_records(coords, reg, sign))
    Z = np.concatenate([p[0] for p in parts])
    GY = np.concatenate([p[1] for p in parts])
    GX = np.concatenate([p[2] for p in parts])
    W0 = np.concatenate([p[3] for p in parts])
    W1 = np.concatenate([p[4] for p in parts])
    YH = np.concatenate([np.broadcast_to(p[5], p[0].shape) for p in parts])
    XH = np.concatenate([np.broadcast_to(p[6], p[0].shape) for p in parts])

    core = Z // SLABS
    pos = Z % SLABS
    q = YH * NH + XH
    gbin = (core * SLABS + pos) * NQ + q
    nbins = GRID * NQ
    counts = np.bincount(gbin, minlength=nbins)
    caps = np.maximum(
        1,
        np.ceil(counts.reshape(CORES, SLABS, NQ).max(axis=0) / 128).astype(np.int64),
    )  # [SLABS, NQ]

    # column offsets per (pos, q), shared by all cores
    flat_caps = caps.reshape(-1)
    col_off = np.zeros(SLABS * NQ, dtype=np.int64)
    np.cumsum(flat_caps[:-1], out=col_off[1:])
    TOT = int(flat_caps.sum())

    order = np.argsort(gbin, kind="stable")
    Z, GY, GX, W0, W1, YH, XH = (a[order] for a in (Z, GY, GX, W0, W1, YH, XH))
    gbin = gbin[order]
    starts = np.zeros(nbins + 1, dtype=np.int64)
    np.cumsum(counts, out=starts[1:])
    rank = np.arange(len(gbin), dtype=np.int64) - starts[gbin]
    core_of = gbin // (SLABS * NQ)
    local_bin = gbin % (SLABS * NQ)
    col = col_off[local_bin] + rank // 128
    part = rank % 128
    dest = (core_of * TOT + col) * 128 + part

    yl = (GY - (YH * WIN + np.float32(WIN / 2 - 0.5))).astype(np.float16)
    xl = (GX - (XH * WIN + np.float32(WIN / 2 - 0.5))).astype(np.float16)

    def field(vals, dtype):
        flat = np.zeros(CORES * TOT * 128, dtype=dtype)
        flat[dest] = vals
        out = []
        for c in range(CORES):
            block = flat[c * TOT * 128:(c + 1) * TOT * 128]
            out.append(np.ascontiguousarray(block.reshape(TOT, 128).T))
        return out

    shards = list(zip(
        field(yl, np.float16),
        field(xl, np.float16),
        field(W0.astype(ml_dtypes.bfloat16), ml_dtypes.bfloat16),
        field(W1.astype(ml_dtypes.bfloat16), ml_dtypes.bfloat16),
    ))
    return shards, caps, col_off, TOT


def _iota_interleaved():
    k = np.arange(WIN, dtype=np.float32) - np.float32(WIN / 2 - 0.5)
    row = np.repeat(k, C)  # iota_i[k*C + j] = k - 31.5
    return np.broadcast_to(row.astype(np.float16)[None, :], (128, WIN * C)).copy()


def _build_program(caps, TOT, dbg=False):
    nc = bacc.Bacc("TRN2", target_bir_lowering=False, debug=False,
                   num_devices=CORES)
    YLd = nc.declare_dram_parameter("YL", [128, TOT], F16, isOutput=False)
    XLd = nc.declare_dram_parameter("XL", [128, TOT], F16, isOutput=False)
    W0d = nc.declare_dram_parameter("W0", [128, TOT], BF16, isOutput=False)
    W1d = nc.declare_dram_parameter("W1", [128, TOT], BF16, isOutput=False)
    IOd = nc.declare_dram_parameter("IOTA", [128, WIN * C], F16, isOutput=False)
    OUTd = nc.declare_dram_parameter("OUT", [128, 2 * SLABS], F32, isOutput=True)
    DBGd = (nc.declare_dram_parameter("DBG", [128, 512 * SLABS], F32,
                                      isOutput=True) if dbg else None)

    AluOp = mybir.AluOpType
    Act = mybir.ActivationFunctionType

    # per-column metadata: (pos, q, idx, cap)
    bmeta = []
    for s in range(SLABS):
        for qq in range(NQ):
            cap = int(caps[s, qq])
            for i in range(cap):
                bmeta.append((s, qq, i, cap))
    assert len(bmeta) == TOT

    def binslice(t, qq):
        yh, xh = qq // NH, qq % NH
        p0 = (yh % PPW) * WIN
        f0 = (yh // PPW) * (NH * WIN) + xh * WIN
        return t[p0:p0 + WIN, f0:f0 + WIN], p0

    with tile.TileContext(nc) as tc:
        with (
            tc.tile_pool(name="persist", bufs=1) as persist,
            tc.tile_pool(name="chunkp", bufs=3) as chunkp,
            tc.tile_pool(name="evac", bufs=2) as evac,
            tc.tile_pool(name="psum", bufs=8, space="PSUM") as psum,
        ):
            yl_t = persist.tile([128, TOT], F16, tag="yl")
            nc.sync.dma_start(out=yl_t[:], in_=YLd[:])
            xl_t = persist.tile([128, TOT], F16, tag="xl")
            nc.sync.dma_start(out=xl_t[:], in_=XLd[:])
            w0_t = persist.tile([128, TOT], BF16, tag="w0")
            nc.sync.dma_start(out=w0_t[:], in_=W0d[:])
            w1_t = persist.tile([128, TOT], BF16, tag="w1")
            nc.sync.dma_start(out=w1_t[:], in_=W1d[:])
            iota_t = persist.tile([128, WIN * C], F16, tag="iota")
            nc.sync.dma_start(out=iota_t[:], in_=IOd[:])
            acc_u = persist.tile([128, SLABS], F32, tag="accu")
            acc_r = persist.tile([128, SLABS], F32, tag="accr")
            zero_t = persist.tile([128, 512], BF16, tag="zero")
            nc.gpsimd.memset(zero_t[:], 0.0)

            ptiles = {}

            def get_ptile(s):
                if s not in ptiles:
                    t = psum.tile([128, NQ * WIN * WIN // 128], F32,
                                  tag="bank", name=f"bank{s}")
                    ptiles[s] = t
                    # full-bank accumulation-group start: pends + zeroes the
                    # whole bank so per-bin matmuls can all accumulate
                    nc.tensor.matmul(t[:], zero_t[:, 0:128], zero_t[:],
                                     start=True, stop=False)
                return ptiles[s]

            def evacuate(s):
                d = ptiles.pop(s)
                # full-bank group stop (accumulates zero)
                nc.tensor.matmul(d[:], zero_t[:, 0:128], zero_t[:],
                                 start=False, stop=True)
                if dbg:
                    dc = evac.tile([128, 512], F32, tag="dbgc")
                    nc.vector.tensor_copy(out=dc[:], in_=d[:])
                    nc.sync.dma_start(out=DBGd[:, s * 512:(s + 1) * 512],
                                      in_=dc[:])
                u = evac.tile([128, 512], BF16, tag="u")
                nc.scalar.activation(out=u[:], in_=d[:], func=Act.Abs)
                squ = evac.tile([128, 512], BF16, tag="squ")
                nc.scalar.activation(
                    out=squ[:], in_=u[:], func=Act.Square,
                    accum_out=acc_u[:, s:s + 1],
                )
                r = evac.tile([128, 512], BF16, tag="r")
                nc.vector.tensor_scalar(
                    out=r[:], in0=u[:], scalar1=1.0, scalar2=0.0,
                    op0=AluOp.subtract, op1=AluOp.max,
                )
                sqr = evac.tile([128, 512], BF16, tag="sqr")
                nc.scalar.activation(
                    out=sqr[:], in_=r[:], func=Act.Square,
                    accum_out=acc_r[:, s:s + 1],
                )

            for cc in range(0, TOT, C):
                cw = min(C, TOT - cc)
                n = cw * WIN

                def iv(t, width=None):
                    """interleaved 3D view [128, WIN, cw] of a chunk tile"""
                    w = width or cw
                    return t[:, :WIN * w].rearrange("p (k j) -> p k j", j=w)

                iota_v = iota_t[:].rearrange("p (k j) -> p k j", j=C)[:, :, :cw]

                def bc(t):
                    return t[:, cc:cc + cw].unsqueeze(1).broadcast_to(
                        (128, WIN, cw))

                da = chunkp.tile([128, WIN * C], BF16, tag="da")
                nc.vector.tensor_tensor(out=iv(da), in0=iota_v, in1=bc(yl_t),
                                        op=AluOp.subtract)
                ea = chunkp.tile([128, WIN * C], BF16, tag="ea")
                nc.scalar.activation(out=ea[:, :n], in_=da[:, :n], func=Act.Abs)
                nty = chunkp.tile([128, WIN * C], BF16, tag="nty")
                nc.vector.tensor_scalar(out=nty[:, :n], in0=ea[:, :n],
                                        scalar1=1.0, scalar2=0.0,
                                        op0=AluOp.subtract, op1=AluOp.min)
                a0 = chunkp.tile([128, WIN * C], BF16, tag="a0")
                nc.vector.tensor_tensor(out=iv(a0), in0=iv(nty), in1=bc(w0_t),
                                        op=AluOp.mult)
                a1 = chunkp.tile([128, WIN * C], BF16, tag="a1")
                nc.gpsimd.tensor_tensor(out=iv(a1), in0=iv(nty), in1=bc(w1_t),
                                        op=AluOp.mult)
                db = chunkp.tile([128, WIN * C], BF16, tag="db")
                nc.vector.tensor_tensor(out=iv(db), in0=iota_v, in1=bc(xl_t),
                                        op=AluOp.subtract)
                ab = chunkp.tile([128, WIN * C], BF16, tag="ab")
                nc.scalar.activation(out=ab[:, :n], in_=db[:, :n], func=Act.Abs)
                ntx = chunkp.tile([128, WIN * C], BF16, tag="ntx")
                nc.vector.tensor_scalar(out=ntx[:, :n], in0=ab[:, :n],
                                        scalar1=1.0, scalar2=0.0,
                                        op0=AluOp.subtract, op1=AluOp.min)

                a0v, a1v, ntxv = iv(a0), iv(a1), iv(ntx)
                for j in range(cw):
                    s, qq, idx, cap = bmeta[cc + j]
                    mov = ntxv[:, :, j]
                    # w0 tap -> slab s (group: prior nxt then these cur)
                    outc, p0 = binslice(get_ptile(s), qq)
                    nc.tensor.matmul(
                        outc, a0v[:, :, j], mov,
                        start=False, stop=False, tile_position=(0, p0),
                    )
                    # w1 tap -> slab s+1 (skipped for pos 31: w1 == 0 there)
                    if s < SLABS - 1:
                        outn, p0n = binslice(get_ptile(s + 1), qq)
                        nc.tensor.matmul(
                            outn, a1v[:, :, j], mov,
                            start=False, stop=False, tile_position=(0, p0n),
                        )
                    if idx == cap - 1 and qq == NQ - 1:
                        evacuate(s)

            nc.sync.dma_start(out=OUTd[:, 0:SLABS], in_=acc_u[:])
            nc.sync.dma_start(out=OUTd[:, SLABS:2 * SLABS], in_=acc_r[:])
    nc.compile()
    return nc


def _get_program():
    return _CACHE["nc"]


def kernel(registration_pred, registration_gt, coords, _trace=False):
    shards, caps, col_off, TOT = _prepare(registration_pred, registration_gt,
                                          coords)
    key = (TOT, caps.tobytes())
    if _CACHE.get("key") != key:
        _CACHE["nc"] = _build_program(caps, TOT)
        _CACHE["key"] = key
    nc = _CACHE["nc"]
    iota = _iota_interleaved()
    in_maps = [
        {"YL": yl, "XL": xl, "W0": w0, "W1": w1, "IOTA": iota}
        for (yl, xl, w0, w1) in shards
    ]
    try:
        res = run_bass_kernel_spmd(nc, in_maps, list(range(CORES)),
                                   trace=_trace)
    except Exception:
        res = run_bass_kernel_spmd(nc, in_maps, list(range(CORES)),
                                   trace=_trace)
    total = 0.0
    for r in res.results:
        out = r["OUT"].astype(np.float64)
        total += 0.5 * (out[:, :SLABS].sum() - out[:, SLABS:].sum())
    if _trace:
        kernel.last_exec_time_ns = res.exec_time_ns
        kernel.last_results = res
    return np.float32(total)
